# revision 1
# baseline (speedup 1.0000x reference)
"""Trainium2 Bass kernel for nn_AttnFathers — PE-streaming edition.

Reference computation:
    energy      = einsum('bmfh,kh->bmfk', FO, W) + bias
    attn_energy = einsum('bh,bmfh->bmf', hidden[0], energy)
    out         = softmax(attn_energy, axis=1)                   # over m

Algebraic rewrite: e[b,r] = FO[b,r,:].v[b] with v = hidden @ W; the bias
term is constant along the softmax axis and cancels.

Why PE: every 1-elem/lane/cycle path (DVE fused stt, ACT reduce) tops
out ~1.3-1.6us per 128x1024 tile and they contend on SBUF ports, capping
the engine side ~150us.  The TensorEngine is idle and has its own SBUF
read ports.  With FO supplied TRANSPOSED by the host (layout change
only), the dot products become tiny matmuls:

    host: FOT[b] = FO[b].reshape(8192, 1024).T        # [H, ROWS] fp16
    per k-chunk (128 rows of FOT = 2 MB): one DMA, then for each of 16
    row-blocks: matmul(out=eT[bid:bid+1, :], lhsT=vT[:,k,b], rhs=chunk)
    accumulating over k in PSUM.  e lands TRANSPOSED: eT[bid, j] =
    e[bid*512 + j], 16 psum partitions x 512.

PE cost: 256 matmuls x (512 cols @2.4GHz + ldweights overhead) ~ 60-95us,
fully hidden under the ~100us DMA stream.  DVE/ACT/Pool only run the
softmax (~6us per batch), so nothing contends with the DMA writes.

Softmax over m on the transposed e (r = m*32+f -> m = 16*bid + j//32,
f = j%32; per (b,f) the softmax spans all partitions x j//32 groups):
    gm   = reduce_max_g(eT)                  # DVE   [16,32]
    amax = partition_all_reduce_max(gm)      # Pool  [16,32] = K_f
    negK = -amax                             # ACT
    eK   = eT + negK (broadcast over g)      # DVE   [16,512]
    pj   = exp(eK)                           # ACT
    s    = reduce_sum_g(pj)                  # DVE   [16,32]
    S    = partition_all_reduce_add(s)       # Pool
    rinv = 1/S                               # DVE
    pn   = pj * rinv (broadcast over g)      # DVE
    out[b, bid*512+j] = pn[bid, j]           # direct DMA, no transpose!

Sharding: data-parallel over batch B=16 -> 2 batches per core on 8 cores.
"""

import sys
import os

for _p in ("/opt/trn_rl_repo", "/root/.axon_site/_ro/trn_rl_repo"):
    if os.path.isdir(_p) and _p not in sys.path:
        sys.path.insert(0, _p)

import numpy as np
from contextlib import ExitStack

import concourse.bass as bass
import concourse.bacc as bacc
import concourse.tile as tile
from concourse import mybir, bass_isa
from concourse.bass_utils import run_bass_kernel_spmd

F32 = mybir.dt.float32
F16 = mybir.dt.float16

B, MAX_LEN, FATHER_NUM, H = 16, 256, 32, 1024
NCORES = 8
BPC = B // NCORES                 # batches per core = 2
ROWS = MAX_LEN * FATHER_NUM       # rows per batch = 8192
P = 128
KC = H // P                       # 128-row chunks of FOT = 8
NBLK = 16                         # 512-row blocks per batch
NJ = ROWS // NBLK                 # 512
GG = NJ // FATHER_NUM             # j//32 groups per block = 16
CHUNK_BUFS = 6                    # in-flight 2MB FOT chunk slices (12 MB)
FILLERS = 8                       # keep-warm matmuls per chunk boundary:
                                  # the PE clock drops 2.4->1.2 GHz after any
                                  # idle gap and takes ~3us to re-ramp. ~1.5us
                                  # of dependency-free matmuls pad the real
                                  # work (3.5us/chunk) up to the DMA chunk
                                  # cadence (~4.7us) so the PE never idles
                                  # long enough to reset.


def build_nc() -> bass.Bass:
    nc = bacc.Bacc(trn_type="TRN2")

    fot = nc.dram_tensor("fot", [BPC, H, ROWS], F16, kind="ExternalInput")
    hidT = nc.dram_tensor("hidT", [H, BPC], F16, kind="ExternalInput")
    w = nc.dram_tensor("w", [H, H], F16, kind="ExternalInput")
    out = nc.dram_tensor("out", [BPC, MAX_LEN, FATHER_NUM], F32, kind="ExternalOutput")

    ident_d = nc.inline_tensor(np.eye(P, dtype=np.float32), "identc")
    # oh[p, bid, c] = 1 if bid == c; scaled by v to build the onehot
    # stationaries (PE psum outputs must start at partition 0, so each
    # block's matmul writes the full [NBLK, NJ] tile with zeros in the
    # other 15 rows).
    oh_np = np.zeros((P, NBLK, NBLK), dtype=np.float16)
    for c in range(NBLK):
        oh_np[:, c, c] = 1.0
    oh_d = nc.inline_tensor(oh_np.reshape(P, NBLK * NBLK), "ohc")
    ones16_d = nc.inline_tensor(np.ones((NBLK, 1), dtype=np.float32), "ones16c")
    ones116_d = nc.inline_tensor(np.ones((1, NBLK), dtype=np.float32), "ones116c")

    with tile.TileContext(nc) as tc, ExitStack() as ctx:
        consts = ctx.enter_context(tc.tile_pool(name="consts", bufs=1))
        wpool = ctx.enter_context(tc.tile_pool(name="wpool", bufs=1))
        chunks = ctx.enter_context(tc.tile_pool(name="chunks", bufs=CHUNK_BUFS))
        smallp = ctx.enter_context(tc.tile_pool(name="smallp", bufs=2))
        outp = ctx.enter_context(tc.tile_pool(name="outp", bufs=2))
        psum1 = ctx.enter_context(tc.tile_pool(name="psum1", bufs=1, space="PSUM"))
        psum_e = ctx.enter_context(tc.tile_pool(name="psum_e", bufs=1, space="PSUM"))
        psum_j = ctx.enter_context(tc.tile_pool(name="psum_j", bufs=1, space="PSUM"))

        # ---- prologue: hidT (tiny) + W, then v = hid @ W, then vT ----------
        hT = consts.tile([P, KC, BPC], F16)
        nc.sync.dma_start(
            out=hT, in_=hidT.ap().rearrange("(k p) b -> p k b", k=KC, p=P)
        )
        wt = wpool.tile([P, KC, H], F16)
        w_ap = w.ap()
        for k in range(KC):
            eng = nc.sync if k % 2 == 0 else nc.scalar
            eng.dma_start(out=wt[:, k, :], in_=w_ap[k * P:(k + 1) * P, :])
        ident = consts.tile([P, P], F32)
        nc.scalar.dma_start(out=ident, in_=ident_d.ap())
        ones16 = consts.tile([NBLK, 1], F32)
        nc.scalar.dma_start(out=ones16, in_=ones16_d.ap())
        ones116 = consts.tile([1, NBLK], F32)
        nc.scalar.dma_start(out=ones116, in_=ones116_d.ap())

        # Warm the ACT exp table and the Pool engine during the prologue.
        warm = consts.tile([1, 1], F32)
        nc.vector.memset(warm, 0.0)
        nc.scalar.activation(
            out=warm, in_=warm, func=mybir.ActivationFunctionType.Exp
        )
        warm2 = consts.tile([1, 1], F32)
        nc.gpsimd.memset(warm2, 0.0)
        nc.gpsimd.partition_all_reduce(
            out_ap=warm2, in_ap=warm2, channels=1,
            reduce_op=bass_isa.ReduceOp.max,
        )

        # v = hid @ W, computed half-by-half so vT/Sv for the first four
        # k-chunks are ready while the second half still accumulates.
        oh = consts.tile([P, NBLK * NBLK], F16)
        nc.scalar.dma_start(out=oh, in_=oh_d.ap())
        v_ps = psum1.tile([BPC, H], F32, tag="vps")
        v_sb = consts.tile([BPC, H], F32)
        vT_ps = psum1.tile([P, KC, BPC], F32, tag="vTps")
        vT = consts.tile([P, KC, BPC], F32)
        Sv = consts.tile([P, KC, BPC, NBLK * NBLK], F16)
        for half in range(2):
            n0, n1 = half * 512, (half + 1) * 512
            for k in range(KC):
                nc.tensor.matmul(
                    v_ps[:, n0:n1], hT[:, k, :], wt[:, k, n0:n1],
                    start=(k == 0), stop=(k == KC - 1),
                )
            nc.vector.tensor_copy(out=v_sb[:, n0:n1], in_=v_ps[:, n0:n1])
            for k in range(4 * half, 4 * half + 4):
                nc.tensor.transpose(
                    vT_ps[:, k, :], v_sb[:, k * P:(k + 1) * P],
                    ident[0:BPC, 0:BPC],
                )
            ks = slice(4 * half, 4 * half + 4)
            nc.vector.tensor_copy(out=vT[:, ks, :], in_=vT_ps[:, ks, :])
            # batch 0's stationaries first: they gate the first real matmul
            for k in range(4 * half, 4 * half + 4):
                nc.vector.tensor_scalar_mul(
                    out=Sv[:, k, 0, :], in0=oh, scalar1=vT[:, k, 0:1]
                )
        for k in range(KC):
            nc.vector.tensor_scalar_mul(
                out=Sv[:, k, 1, :], in0=oh, scalar1=vT[:, k, 1:2]
            )

        # ---- main stream: per batch, 8 chunk-DMAs x 16 block-matmuls -------
        # FOT row h = k*128+p; chunk k = rows [k*128,(k+1)*128) = [128, 8192]
        # fp16 (2 MB, 16KB per partition, fully contiguous lines).
        fot_r = fot.ap().rearrange("b (k p) r -> b k p r", k=KC, p=P)
        out_r = (
            out.ap()
            .rearrange("b m f -> b (m f)")
            .rearrange("b (c j) -> b c j", c=NBLK, j=NJ)
        )

        eT = []
        for b in range(BPC):
            eT_b = psum_e.tile([NBLK, NJ], F32, tag=f"eT{b}")
            eT.append(eT_b)

        # Dependency-free keep-warm matmul: resident Sv x resident wt into a
        # junk psum bank nobody reads.  Runs whenever the PE would otherwise
        # idle at a chunk boundary, holding the clock at 2.4 GHz.
        junk_ps = psum_j.tile([NBLK, 512], F32, tag="junk")

        def fillers(n=FILLERS):
            # operands are wt-only so fillers have no Sv dependency and can
            # also bridge the prologue -> first-chunk window
            for _ in range(n):
                nc.tensor.matmul(
                    junk_ps[:, :], wt[:, 0, 0:NBLK], wt[:, 0, 0:512],
                    start=True, stop=True, skip_group_check=True,
                )

        # bridge the PE from the prologue matmuls to the first chunk
        fillers(12)

        def emit_chunk(b, k, trailing_fillers=True):
            # Split the 2MB chunk DMA into two 1MB halves on the same ring:
            # subtile dependency tracking lets blocks 0-7 start as soon as
            # the first half lands, halving the PE's wait granularity.
            ck = chunks.tile([P, ROWS], F16, tag="ck")
            eng = nc.sync if (b * KC + k) % 2 == 0 else nc.scalar
            eng.dma_start(out=ck[:, 0:ROWS // 2], in_=fot_r[b, k][:, 0:ROWS // 2])
            eng.dma_start(out=ck[:, ROWS // 2:], in_=fot_r[b, k][:, ROWS // 2:])
            for half in range(2):
                for bid in range(8 * half, 8 * half + 8):
                    nc.tensor.matmul(
                        eT[b][:, :],
                        Sv[:, k, b, bid * NBLK:(bid + 1) * NBLK],
                        ck[:, bid * NJ:(bid + 1) * NJ],
                        start=(k == 0 and bid == 0),
                        stop=(k == KC - 1 and bid == NBLK - 1),
                        skip_group_check=True,
                    )
                # no steady-state fillers: with 1MB arrival granularity
                # the PE's worst-case wait (~2.3us) stays under the 3us
                # clock-reset threshold, so keep-warm padding is pure
                # overhead here (only the prologue bridge fillers remain).
                pass

        def emit_batch(b):
            for k in range(KC):
                emit_chunk(b, k)

        def softmax_segments(b):
            st = {}
            eT_b = eT[b]
            eT_gf = eT_b[:, :].rearrange("p (g f) -> p f g", g=GG, f=FATHER_NUM)

            def seg1():  # per-(bid,f) max over g (DVE), cross-bid max (Pool)
                gm = smallp.tile([NBLK, FATHER_NUM], F32, tag="gm")
                nc.vector.tensor_reduce(
                    out=gm, in_=eT_gf, axis=mybir.AxisListType.X,
                    op=mybir.AluOpType.max,
                )
                amax = smallp.tile([NBLK, FATHER_NUM], F32, tag="amax")
                nc.gpsimd.partition_all_reduce(
                    out_ap=amax, in_ap=gm, channels=NBLK,
                    reduce_op=bass_isa.ReduceOp.max,
                )
                st["amax"] = amax

            def seg2():  # eK = eT - K_f directly (DVE subtract w/ broadcast)
                eK = outp.tile([NBLK, NJ], F32, tag="eK")
                amax_bc = st["amax"][:, :].unsqueeze(1).broadcast_to(
                    [NBLK, GG, FATHER_NUM]
                )
                nc.vector.tensor_tensor(
                    out=eK.rearrange("p (g f) -> p g f", g=GG, f=FATHER_NUM),
                    in0=eT_b[:, :].rearrange(
                        "p (g f) -> p g f", g=GG, f=FATHER_NUM
                    ),
                    in1=amax_bc, op=mybir.AluOpType.subtract,
                )
                st["eK"] = eK

            def seg3():  # exp (ACT), per-f partial sums (DVE)
                pj = outp.tile([NBLK, NJ], F32, tag="pj")
                nc.scalar.activation(
                    out=pj, in_=st["eK"],
                    func=mybir.ActivationFunctionType.Exp,
                )
                s = smallp.tile([NBLK, FATHER_NUM], F32, tag="s")
                nc.vector.tensor_reduce(
                    out=s,
                    in_=pj[:, :].rearrange(
                        "p (g f) -> p f g", g=GG, f=FATHER_NUM
                    ),
                    axis=mybir.AxisListType.X, op=mybir.AluOpType.add,
                )
                st["pj"] = pj
                st["s"] = s

            def seg4():  # cross-bid sum + broadcast on PE (faster than Pool),
                         # 1/S (DVE), scale + store
                S_ps = psum1.tile([1, FATHER_NUM], F32, tag="Sps")
                nc.tensor.matmul(S_ps, ones16, st["s"], start=True, stop=True)
                rinv = smallp.tile([1, FATHER_NUM], F32, tag="rinv")
                nc.vector.reciprocal(out=rinv, in_=S_ps)
                rbc_ps = psum1.tile([NBLK, FATHER_NUM], F32, tag="rbc")
                nc.tensor.matmul(rbc_ps, ones116, rinv, start=True, stop=True)
                pn = outp.tile([NBLK, NJ], F32, tag="pn")
                rinv_bc = rbc_ps[:, :].unsqueeze(1).broadcast_to(
                    [NBLK, GG, FATHER_NUM]
                )
                nc.vector.tensor_tensor(
                    out=pn.rearrange("p (g f) -> p g f", g=GG, f=FATHER_NUM),
                    in0=st["pj"][:, :].rearrange(
                        "p (g f) -> p g f", g=GG, f=FATHER_NUM
                    ),
                    in1=rinv_bc, op=mybir.AluOpType.mult,
                )
                nc.scalar.dma_start(out=out_r[b], in_=pn)

            return [seg1, seg2, seg3, seg4]

        emit_batch(0)
        segs0 = softmax_segments(0)
        # Batch 0's softmax interleaves into batch 1's PE stream (it only
        # uses DVE/ACT/Pool, which are idle during the stream).
        for k in range(KC):
            emit_chunk(1, k, trailing_fillers=(k < KC - 1))
            if 1 <= k <= 4:
                segs0[k - 1]()
        for seg in softmax_segments(1):
            seg()

    nc.compile()
    return nc


_NC_CACHE = None


def _get_nc():
    global _NC_CACHE
    if _NC_CACHE is None:
        _NC_CACHE = build_nc()
    return _NC_CACHE


def _make_in_maps(hidden, fathers_outputs, attn_W, attn_b):
    hidden = np.asarray(hidden, dtype=np.float32)
    fo16 = np.asarray(fathers_outputs, dtype=np.float32).astype(np.float16)
    # host-side transpose: FOT[b] = FO[b].reshape(ROWS, H).T  -> [H, ROWS]
    fot = np.ascontiguousarray(
        fo16.reshape(B, ROWS, H).transpose(0, 2, 1)
    )
    w16 = np.ascontiguousarray(np.asarray(attn_W, dtype=np.float32).astype(np.float16))
    in_maps = []
    for i in range(NCORES):
        b0 = i * BPC
        in_maps.append({
            "fot": np.ascontiguousarray(fot[b0:b0 + BPC]),
            "hidT": np.ascontiguousarray(
                hidden[0, b0:b0 + BPC].T.astype(np.float16)
            ),
            "w": w16,
        })
    return in_maps


def run(hidden, fathers_outputs, fathers_lengths, attn_W, attn_b, trace=False):
    """Run on the 8 NeuronCores; returns (full_output, BassKernelResults)."""
    nc = _get_nc()
    in_maps = _make_in_maps(hidden, fathers_outputs, attn_W, attn_b)
    res = run_bass_kernel_spmd(nc, in_maps, list(range(NCORES)), trace=trace)
    parts = [np.asarray(res.results[i]["out"]) for i in range(NCORES)]
    full = np.concatenate(parts, axis=0).astype(np.float32)
    return full, res


def kernel(hidden, fathers_outputs, fathers_lengths, attn_W, attn_b):
    full, _ = run(hidden, fathers_outputs, fathers_lengths, attn_W, attn_b)
    return full

